# revision 54
# baseline (speedup 1.0000x reference)
"""Trainium2 Bass kernel for nn_NeuralControlActor (batch-1 MLP + 4x Mamba2 + MLP).

Tensor-parallel over 8 NeuronCores:
  - mlp_in W1/W2 row-sharded (+AllGather), Mamba2 heads sharded 4/core
    (B/C/conv replicated), gated-RMSNorm stat + Wout partial fused into one
    AllReduce per layer, mlp_out w3/w4 row-sharded (+AllGather).
  - The T=32 sequential scan is computed in closed form (SSD):
    Y_h = (exp(S_t - S_s) o (B^T C) o dt_s + D_h I)^T @ X_h  -- all matmuls.
  - Weights stream as bf16 (host-cast); SSD/norm math stays f32.

Self-contained: hardcodes all shapes; host prep is pure numpy.
"""
import numpy as np
import ml_dtypes

S, H, T, NL = 4096, 1024, 32, 4
DI, DS, DC, HD, NH = 2048, 128, 4, 64, 32
CD = DI + 2 * DS
DIP = 2 * DI + 2 * DS + NH
ASIZE = 16384
NCORES = 8
HC = NH // NCORES            # 4 heads/core
XC = HC * HD                 # 256 x/z channels per core
MASK_POS = 1000.0
BF16 = ml_dtypes.bfloat16
FP8 = ml_dtypes.float8_e4m3

# ----------------------------------------------------------------- host prep

def _chunkT(w):
    """w [F, K] -> arr [128, K//128, F]: arr[p, k, f] = w[f, k*128+p]."""
    F, K = w.shape
    kc = K // 128
    return np.ascontiguousarray(w.T.reshape(kc, 128, F).transpose(1, 0, 2))


def _prep_consts():
    f32 = np.float32
    c = {}
    ut = np.triu(np.ones((T, T), f32))
    c["negut"] = np.ascontiguousarray(-ut)
    maskpos = (np.tril(np.ones((T, T), f32), -1) * MASK_POS).astype(f32)
    # batched-SSD consts (bf16 block): ones4, -maskpos^T, blockdiag masks
    ones4 = np.zeros((T, T), f32)
    ones4[0:HC] = 1.0
    c["ones4"] = ones4
    c["negmaskT"] = np.ascontiguousarray(-maskpos.T)
    blkpos = np.kron(np.eye(HC, dtype=f32), np.ones((1, T), f32))  # [4,128]
    bp = np.zeros((T, HC * T), f32)
    bp[0:HC] = blkpos
    c["blkpos"] = bp
    bn = np.zeros((T, HC * T), f32)
    bn[0:HC] = -blkpos
    c["blkneg"] = bn
    c["blk32"] = np.tile(np.eye(T, dtype=f32), (1, HC))            # [32,128]
    c["ones1"] = np.ones((1, 128), f32)
    return c


def _prep_core(inp, c, wdt):
    f32 = np.float32
    m = {}
    x = np.asarray(inp["x"], f32)
    # x8[p, i, j] = x[(2j+i)*128 + p] -- paired k-chunks for DoubleRow
    m["x8"] = np.ascontiguousarray(
        x.reshape(16, 2, 128).transpose(2, 1, 0)).astype(FP8)    # [128,2,16]

    w1c = np.asarray(inp["w1"], f32)[c * 128:(c + 1) * 128]
    w1t = _chunkT(w1c).astype(FP8)                               # [128,32,128]
    m["w1dr"] = np.ascontiguousarray(w1t.reshape(128, 16, 2, 128))
    m["b1row"] = np.asarray(inp["b1"], f32)[None, c * 128:(c + 1) * 128]

    w2c = np.asarray(inp["w2"], f32)[c * 4096:(c + 1) * 4096]
    w2t = _chunkT(w2c)                                           # [128,8,4096]
    m["w2dr"] = np.ascontiguousarray(
        w2t.reshape(128, 8, 8, 512).transpose(2, 0, 1, 3)
        .reshape(8, 128, 4, 2, 512)).astype(FP8)
    m["b2row"] = np.asarray(inp["b2"], f32)[None, c * 4096:(c + 1) * 4096]

    for l in range(NL):
        win = np.asarray(inp["m_Win"], f32)[l]
        zrows = win[c * XC:(c + 1) * XC]
        dtrows = win[2 * DI + 2 * DS + c * HC:2 * DI + 2 * DS + (c + 1) * HC]
        m[f"winzd{l}"] = _chunkT(np.concatenate([zrows, dtrows], 0)).astype(FP8)
        xrows = win[DI + c * XC:DI + (c + 1) * XC]
        brows = win[2 * DI:2 * DI + DS]
        crows = win[2 * DI + DS:2 * DI + 2 * DS]
        m[f"winxbc{l}"] = _chunkT(
            np.concatenate([xrows, brows, crows], 0)).astype(FP8)  # [128,8,512]

        cw = np.asarray(inp["m_convw"], f32)[l]
        cb = np.asarray(inp["m_convb"], f32)[l]
        chsel = np.concatenate([
            np.arange(c * XC, (c + 1) * XC),
            np.arange(DI, DI + DS),
            np.arange(DI + DS, DI + 2 * DS)])
        m[f"convw{l}"] = np.ascontiguousarray(
            cw[chsel].reshape(4, 128, 4).transpose(1, 0, 2))     # [128,4,4]
        m[f"convb{l}"] = np.ascontiguousarray(cb[chsel].reshape(4, 128).T)

        dtb = np.asarray(inp["m_dtbias"], f32)[l, c * HC:(c + 1) * HC]
        m[f"dtbias{l}"] = np.broadcast_to(dtb[None, :], (T, HC)).copy()
        A = -np.exp(np.asarray(inp["m_Alog"], f32)[l, c * HC:(c + 1) * HC])
        m[f"abc{l}"] = np.broadcast_to(A[None, :], (T, HC)).copy()
        Dv = np.asarray(inp["m_D"], f32)[l, c * HC:(c + 1) * HC]
        dd = np.zeros((T, HC, T), f32)
        for h in range(HC):
            dd[:, h, :] = np.eye(T, dtype=f32) * Dv[h]
        m[f"ddiag{l}"] = dd

        woutp = (np.asarray(inp["m_Wout"], f32)[l]
                 * np.asarray(inp["m_normw"], f32)[l][None, :])
        wc = woutp[:, c * XC:(c + 1) * XC]
        m[f"wout{l}"] = np.ascontiguousarray(
            wc.T.reshape(2, 128, 8, 128).transpose(1, 0, 2, 3)).astype(FP8)

    w3c = np.asarray(inp["w3"], f32)[c * 128:(c + 1) * 128]
    w3t = _chunkT(w3c)                                           # [128,256,128]
    # pair chunk j with chunk j+128 (first/second half of flat input)
    m["w3dr"] = np.ascontiguousarray(
        w3t.reshape(128, 2, 8, 16, 128).transpose(2, 0, 3, 1, 4)).astype(FP8)
    m["b3row"] = np.asarray(inp["b3"], f32)[None, c * 128:(c + 1) * 128]
    w4c = np.asarray(inp["w4"], f32)[c * 2048:(c + 1) * 2048]
    w4t = _chunkT(w4c)                                           # [128,8,2048]
    m["w4dr"] = np.ascontiguousarray(
        w4t.reshape(128, 8, 4, 512).transpose(2, 0, 1, 3)
        .reshape(4, 128, 4, 2, 512)).astype(FP8)
    m["b4row"] = np.asarray(inp["b4"], f32)[None, c * 2048:(c + 1) * 2048]
    m["g4"] = np.ascontiguousarray(
        np.asarray(inp["b4"], f32)[c * 2048:(c + 1) * 2048].reshape(4, 512))

    # packed consts: g32 [32, W] f32, g32b [32, W] bf16, g1 [1, W], gcw [128, W]
    cst = _prep_consts()
    g32 = [cst["negut"]]
    for l in range(NL):
        g32 += [m.pop(f"dtbias{l}"), m.pop(f"abc{l}"),
                m.pop(f"ddiag{l}").reshape(T, HC * T)]
    m["g32"] = np.concatenate(g32, 1)                       # [32, 576]
    m["g32b"] = np.concatenate(
        [cst["ones4"], cst["negmaskT"], cst["blkpos"], cst["blkneg"],
         cst["blk32"]], 1).astype(wdt)                      # [32, 448]
    g1 = [cst["ones1"], m.pop("b1row"), m.pop("b3row"),
          m.pop("b2row"), m.pop("b4row")]
    m["g1"] = np.concatenate(g1, 1).astype(wdt)            # [1, 6528]
    gcw = []
    for l in range(NL):
        gcw += [m.pop(f"convw{l}").reshape(128, 16), m.pop(f"convb{l}")]
    m["gcw"] = np.concatenate(gcw, 1)                       # [128, 80]
    return m


# ------------------------------------------------------------- bass program

def _build_program(wdt_np, collectives=True):
    from contextlib import ExitStack
    import concourse.bacc as bacc
    import concourse.tile as tile
    import concourse.bass as bass
    from concourse import mybir
    from concourse.masks import make_identity

    f32 = mybir.dt.float32
    wdt = mybir.dt.from_np(np.dtype(wdt_np))
    dt8 = mybir.dt.float8e4
    Alu = mybir.AluOpType
    Act = mybir.ActivationFunctionType

    # Force a single ACT table set: every function this kernel uses
    # (exp, ln, relu, square, copy) lives in natural_log_exp_and_others;
    # the default chooser thrashes between exp-only and ln-only sets.
    if not getattr(bacc, "_act_tables_pinned", False):
        _orig_gat = bacc.get_activation_tables

        def _gat(arch):
            t = _orig_gat(arch)
            keep = "natural_log_exp_and_others"
            if keep in t:
                for k in t:
                    if k != keep:
                        t[k] = set()
            return t

        bacc.get_activation_tables = _gat
        bacc._act_tables_pinned = True

    nc = bacc.Bacc("TRN2", target_bir_lowering=False, debug=False,
                   num_devices=NCORES)

    def din(name, shape, dt=wdt):
        return nc.dram_tensor(name, list(shape), dt, kind="ExternalInput").ap()

    # inputs (names must match the per-core map)
    x8_d = din("x8", [128, 2, 16], dt8)
    w1t_d = din("w1dr", [128, 16, 2, 128], dt8)
    w2t_d = din("w2dr", [8, 128, 4, 2, 512], dt8)
    layer_d = []
    for l in range(NL):
        layer_d.append(dict(
            winzd=din(f"winzd{l}", [128, 8, 260], dt8),
            winxbc=din(f"winxbc{l}", [128, 8, 512], dt8),
            wout=din(f"wout{l}", [128, 2, 8, 128], dt8),
        ))
    g32_d = din("g32", [T, 576], f32)
    g32b_d = din("g32b", [T, 448])
    g1_d = din("g1", [1, 6528])
    g4_d = din("g4", [4, 512], f32)
    gcw_d = din("gcw", [128, 80], f32)
    w3t_d = din("w3dr", [8, 128, 16, 2, 128], dt8)
    w4t_d = din("w4dr", [4, 128, 4, 2, 512], dt8)
    out_d = nc.dram_tensor("out", [1, 2048], f32, kind="ExternalOutput").ap()
    DR = mybir.MatmulPerfMode.DoubleRow

    RG = [list(range(NCORES))]

    def _collective(kind, op, ins, outs):
        if collectives:
            nc.gpsimd.collective_compute(kind, op, replica_groups=RG,
                                         ins=ins, outs=outs)
        else:
            nc.sync.dma_start(out=outs[0][0:1], in_=ins[0][0:1])

    from concourse.tile import add_dep_helper

    with tile.TileContext(nc) as tc, ExitStack() as ctx:
        ep = ctx.enter_context
        consts = ep(tc.tile_pool(name="consts", bufs=1))
        pw1 = ep(tc.tile_pool(name="pw1", bufs=1))
        pw2 = ep(tc.tile_pool(name="pw2", bufs=8))
        pwin = ep(tc.tile_pool(name="pwin", bufs=4))
        pw3 = ep(tc.tile_pool(name="pw3", bufs=8))
        pw4 = ep(tc.tile_pool(name="pw4", bufs=4))
        pact = ep(tc.tile_pool(name="pact", bufs=2))
        pact1 = ep(tc.tile_pool(name="pact1", bufs=1))
        psm = ep(tc.tile_pool(name="psm", bufs=2))
        pmv = ep(tc.tile_pool(name="pmv", bufs=2, space="PSUM"))
        pps = ep(tc.tile_pool(name="pps", bufs=2, space="PSUM"))
        ppt = ep(tc.tile_pool(name="ppt", bufs=2, space="PSUM"))
        pyacc = ep(tc.tile_pool(name="pyacc", bufs=1, space="PSUM"))
        dram = ep(tc.tile_pool(name="dram", bufs=2, space="DRAM"))

        def sb(pool, shape, dt=f32, tag=None):
            return pool.tile(list(shape), dt, tag=tag, name=tag)

        # ---- stage-A inputs first: every core's first-collective trigger
        # waits on these
        x8 = sb(consts, [128, 2, 16], dt8, tag="x8")
        nc.sync.dma_start(out=x8, in_=x8_d)
        w1sb = sb(pw1, [128, 16, 2, 128], dt8, tag="w1")
        nc.sync.dma_start(out=w1sb, in_=w1t_d)
        # ---- constants into SBUF
        eps_t = sb(consts, [128, 1], f32, tag="eps_t")
        nc.vector.memset(eps_t, 1e-5)
        idn = sb(consts, [128, 128], f32, tag="idn")
        make_identity(nc, idn)
        idnb = sb(consts, [128, 128], wdt, tag="idnb")
        nc.vector.tensor_copy(out=idnb, in_=idn)
        onesr = sb(consts, [1, 128], f32, tag="onesr")
        nc.vector.memset(onesr, 1.0)
        g32 = sb(consts, [T, 576], f32, tag="g32")
        nc.sync.dma_start(out=g32, in_=g32_d)
        g32b = sb(consts, [T, 448], wdt, tag="g32b")
        nc.sync.dma_start(out=g32b, in_=g32b_d)
        g1 = sb(consts, [1, 6528], wdt, tag="g1")
        nc.sync.dma_start(out=g1, in_=g1_d)
        g4 = sb(consts, [4, 512], f32, tag="g4")
        nc.sync.dma_start(out=g4, in_=g4_d)
        gcw = sb(consts, [128, 80], f32, tag="gcw")
        nc.sync.dma_start(out=gcw, in_=gcw_d)
        negut = g32[:, 0:T]
        ones1 = onesr
        onescol = sb(consts, [128, 1], f32, tag="onescol")
        nc.vector.memset(onescol, 1.0)
        ones4 = g32b[0:HC, 0:T]
        negmaskT = g32b[:, T:2 * T]
        blkpos = g32b[0:HC, 2 * T:2 * T + HC * T]
        blkneg = g32b[0:HC, 2 * T + HC * T:2 * T + 2 * HC * T]
        blk32 = g32b[:, 2 * T + 2 * HC * T:2 * T + 3 * HC * T]
        b1row = g1[0:1, 128:256]
        b3row = g1[0:1, 256:384]
        b2row = g1[0:1, 384:4480]
        b4row = g1[0:1, 4480:6528]
        lc = []
        for l in range(NL):
            base = T + l * (2 * HC + HC * T)
            lc.append(dict(
                dtbias=g32[:, base:base + HC],
                abc=g32[:, base + HC:base + 2 * HC],
                ddiag=g32[:, base + 2 * HC:base + 2 * HC + HC * T],
                convw=gcw[:, l * 20:l * 20 + 16],
                convb=gcw[:, l * 20 + 16:l * 20 + 20],
            ))

        # ---- stage A: h = relu(W1 x + b1), row shard -> AllGather
        ps_h = pmv.tile([1, 512], f32, tag="mv", name="mv")
        for j in range(16):
            nc.tensor.matmul(ps_h[0:1, 0:128], x8[:, :, j:j + 1], w1sb[:, j],
                             start=(j == 0), stop=(j == 15), perf_mode=DR)
        h_tmp = sb(pact, [1, 128], f32, tag="h_tmp")
        i_gate_a = nc.vector.tensor_tensor(h_tmp, ps_h[0:1, 0:128], b1row, Alu.add)
        h_act = sb(pact, [1, 128], wdt, tag="h_act")
        nc.scalar.activation(h_act, h_tmp, Act.Relu)
        hin = dram.tile([1, 128], wdt, tag="hin", name="hin")
        i_hin = nc.sync.dma_start(out=hin[:], in_=h_act, single_packet=True)
        hout = dram.tile([NCORES, 128], wdt, tag="hout", name="hout")
        _collective("AllGather", Alu.bypass, [hin[:].opt()], [hout[:].opt()])

        # HAM keep-warm: dummy fp8 matmuls gated on a pacing instruction so
        # the PE stays above the MID-window threshold through barrier +
        # collective waits (else it re-throttles to 1.2 GHz).
        def keep_warm(n, gate_ins, label):
            psf = pmv.tile([1, 512], f32, tag="mv", name="mv")
            for i in range(n):
                im = nc.tensor.matmul(psf, x8[:, 0, 0:1], w1sb[:, 0:2],
                                      start=True, stop=True)
                if i == 0 and gate_ins is not None:
                    add_dep_helper(im.ins, gate_ins.ins,
                                   reason=f"warm {label}")

        keep_warm(190, i_hin, "AG0")
        h_all = sb(pact, [128, NCORES], wdt, tag="h_all")
        nc.sync.dma_start(out=h_all, in_=hout[:].rearrange("j p -> p j"),
                          single_packet=True)
        h8c = sb(pact, [128, 2, 16], dt8, tag="h8c")
        h_all_pair = bass.AP(tensor=h_all.tensor, offset=h_all.offset,
                             ap=[list(h_all.ap[0]), [1, 2], [2, 4]])
        nc.vector.tensor_copy(out=h8c[:, :, 0:4], in_=h_all_pair)

        # ---- stage B: seq shard = W2 h + b2 (4 tokens) -> AllGather
        seq_sb = sb(pact1, [1, 4096], wdt, tag="seq_sb")
        for nt in range(8):
            w2sb = sb(pw2, [128, 4, 2, 512], dt8, tag="w2")
            i_d = nc.scalar.dma_start(out=w2sb, in_=w2t_d[nt])
            add_dep_helper(i_d.ins, i_gate_a.ins, reason="pace w2 after stage A")
            ps = pmv.tile([1, 512], f32, tag="mv", name="mv")
            for j in range(4):
                nc.tensor.matmul(ps, h8c[:, :, j:j + 1], w2sb[:, j],
                                 start=(j == 0), stop=(j == 3), perf_mode=DR)
            nc.vector.tensor_tensor(seq_sb[0:1, nt * 512:(nt + 1) * 512], ps,
                                    b2row[0:1, nt * 512:(nt + 1) * 512], Alu.add)
        seqin = dram.tile([1, 4096], wdt, tag="seqin", name="seqin")
        i_seqin = nc.sync.dma_start(out=seqin[:], in_=seq_sb,
                                    single_packet=True)
        seqout = dram.tile([NCORES, 4096], wdt, tag="seqout", name="seqout")
        _collective("AllGather", Alu.bypass, [seqin[:].opt()], [seqout[:].opt()])
        keep_warm(30, i_seqin, "AG1")

        # ---- layer-0 input: load [32,1024] token-major, PE-transpose to uT
        useq = sb(pact1, [T, 1024], wdt, tag="useq")
        i_gate_l0 = nc.sync.dma_start(
            out=useq, in_=seqout[:].rearrange("j (a f) -> (j a) f", a=4))
        uTb = sb(pact, [128, 8, T], wdt, tag="uTb")
        for k in range(8):
            pt = pps.tile([128, T], wdt, tag="t128b", name="t128b", bufs=1)
            nc.tensor.transpose(pt, useq[:, k * 128:(k + 1) * 128],
                                idnb[0:T, 0:T])
            nc.vector.tensor_copy(out=uTb[:, k, :], in_=pt)

        # ---- Mamba2 layers
        # Post-AR, the projections contract the UNSCALED partial sum (the
        # gated-RMSNorm scale r is per-token, so it commutes with the
        # channel contraction); r folds into the dt bias-add, the conv
        # input copies, and the z gate -- the norm chain runs in parallel.
        comb2 = None
        for l in range(NL):
            ld, cl = layer_d[l], lc[l]
            winzd = sb(pwin, [128, 8, 260], dt8, tag="winzd")
            i_d = nc.scalar.dma_start(out=winzd, in_=ld["winzd"])
            add_dep_helper(i_d.ins, i_gate_a.ins, reason="pace win after stage A")
            winxbc = sb(pwin, [128, 8, 512], dt8, tag="winxbc")
            i_d = nc.scalar.dma_start(out=winxbc, in_=ld["winxbc"])
            add_dep_helper(i_d.ins, i_gate_a.ins, reason="pace win after stage A")
            woutsb = sb(pwin, [128, 2, 8, 128], dt8, tag="wout")
            i_d = nc.scalar.dma_start(out=woutsb, in_=ld["wout"])
            add_dep_helper(i_d.ins, i_gate_a.ins, reason="pace win after stage A")

            if l == 0:
                u_mm = [uTb[:, k, :] for k in range(8)]
                r_col = r_bc = None
            else:
                u_mm = [comb2[:, k * T:(k + 1) * T] for k in range(8)]
                # norm scale r from the sumsq stat (parallel to projections)
                s_col = sb(pact, [T, 1], f32, tag="s_col")
                nc.scalar.activation(s_col, comb2[0:T, 256:257], Act.Ln,
                                     bias=eps_t[0:T], scale=1.0 / DI)
                r_col = sb(pact, [T, 1], f32, tag="r_col")
                nc.scalar.activation(r_col, s_col, Act.Exp, scale=-0.5)
                rt_ps = ppt.tile([1, T], f32, tag="t256", name="t256")
                nc.tensor.transpose(rt_ps, r_col, idn[0:T, 0:T])
                r_row = sb(pact, [1, T], f32, tag="r_row")
                nc.vector.tensor_copy(out=r_row, in_=rt_ps)
                ps_r = pps.tile([128, T], f32, tag="t128", name="t128")
                nc.tensor.matmul(ps_r, ones1, r_row, start=True, stop=True)
                r_bc = sb(pact, [128, T], f32, tag="r_bc")
                nc.vector.tensor_copy(out=r_bc, in_=ps_r)

            # z feature-major [128, 2T] + dt token-major [T, 4]
            ps_z = ppt.tile([128, 2 * T], f32, tag="t256", name="t256")
            for half in range(2):
                for k in range(8):
                    nc.tensor.matmul(ps_z[:, half * T:(half + 1) * T],
                                     winzd[:, k, half * 128:(half + 1) * 128],
                                     u_mm[k],
                                     start=(k == 0), stop=(k == 7))
            ps_dt = pps.tile([T, HC], f32, tag="t128", name="t128")
            for k in range(8):
                nc.tensor.matmul(ps_dt, u_mm[k], winzd[:, k, 256:260],
                                 start=(k == 0), stop=(k == 7))
            if l == 0:
                dtt = sb(psm, [T, HC], f32, tag="dtt")
                nc.vector.tensor_tensor(dtt, ps_dt, cl["dtbias"], Alu.add)
                z_s = ps_z
            else:
                dtt = sb(psm, [T, HC], f32, tag="dtt")
                nc.vector.scalar_tensor_tensor(dtt, ps_dt, r_col,
                                               cl["dtbias"],
                                               op0=Alu.mult, op1=Alu.add)
                z_s = sb(pact, [128, 2 * T], f32, tag="z_s")
                r_bcz = bass.AP(tensor=r_bc.tensor, offset=r_bc.offset,
                                ap=[list(r_bc.ap[0]), [0, 2], [1, T]])
                nc.vector.tensor_tensor(z_s, ps_z, r_bcz, Alu.mult)
            # silu(z) = z * exp(-ln(1+exp(-z))), feature-major
            zeg = sb(pact, [128, 2 * T], f32, tag="zeg")
            nc.scalar.activation(zeg, z_s, Act.Exp, scale=-1.0)
            zsp = sb(pact, [128, 2 * T], f32, tag="zsp")
            nc.scalar.activation(zsp, zeg, Act.Ln, bias=1.0)
            zsg = sb(pact, [128, 2 * T], f32, tag="zsg")
            nc.scalar.activation(zsg, zsp, Act.Exp, scale=-1.0)
            szf = sb(pact, [128, 2 * T], f32, tag="szf")
            nc.vector.tensor_tensor(szf, z_s, zsg, Alu.mult)
            # x/B/C feature-major, batched causal conv + silu
            xpad = pact.tile([128, 4, 3 + T], f32, tag="xpad", name="xpad")
            nc.vector.memset(xpad[:, :, 0:3], 0.0)
            for ft in range(4):
                ps_x = pps.tile([128, T], f32, tag="t128", name="t128")
                for k in range(8):
                    nc.tensor.matmul(
                        ps_x, winxbc[:, k, ft * 128:(ft + 1) * 128],
                        u_mm[k], start=(k == 0), stop=(k == 7))
                if l == 0:
                    nc.vector.tensor_copy(out=xpad[:, ft, 3:3 + T], in_=ps_x)
                else:
                    nc.vector.tensor_tensor(xpad[:, ft, 3:3 + T], ps_x, r_bc,
                                            Alu.mult)

            def cwb(k):
                a = cl["convw"]
                return bass.AP(tensor=a.tensor, offset=a.offset + k,
                               ap=[list(a.ap[0]), [4, 4], [0, T]])
            c0 = sb(pact, [128, 4, T], f32, tag="cc0")
            nc.vector.tensor_tensor(c0, xpad[:, :, 0:T], cwb(0), Alu.mult)
            c1 = sb(pact, [128, 4, T], f32, tag="cc1")
            nc.gpsimd.tensor_tensor(c1, xpad[:, :, 1:1 + T], cwb(1), Alu.mult)
            c2 = sb(pact, [128, 4, T], f32, tag="cc2")
            nc.vector.tensor_tensor(c2, xpad[:, :, 2:2 + T], cwb(2), Alu.mult)
            c3 = sb(pact, [128, 4, T], f32, tag="cc3")
            nc.gpsimd.tensor_tensor(c3, xpad[:, :, 3:3 + T], cwb(3), Alu.mult)
            c01 = sb(pact, [128, 4, T], f32, tag="cc01")
            nc.vector.tensor_tensor(c01, c0, c1, Alu.add)
            cb_b = bass.AP(tensor=cl["convb"].tensor,
                           offset=cl["convb"].offset,
                           ap=[list(cl["convb"].ap[0]), [1, 4], [0, T]])
            c23 = sb(pact, [128, 4, T], f32, tag="cc23")
            nc.gpsimd.tensor_tensor(c23, c2, c3, Alu.add)
            cpre = sb(pact, [128, 4, T], f32, tag="cpre")
            nc.vector.tensor_tensor(cpre, c01, cb_b, Alu.add)
            conv = sb(pact, [128, 4, T], f32, tag="conv")
            nc.vector.tensor_tensor(conv, cpre, c23, Alu.add)
            xeg = sb(pact, [128, 4, T], f32, tag="xeg")
            nc.scalar.activation(xeg, conv, Act.Exp, scale=-1.0)
            xsp = sb(pact, [128, 4, T], f32, tag="xsp")
            nc.scalar.activation(xsp, xeg, Act.Ln, bias=1.0)
            xsg = sb(pact, [128, 4, T], f32, tag="xsg")
            nc.scalar.activation(xsg, xsp, Act.Exp, scale=-1.0)
            sxbc = sb(pact, [128, 4, T], f32, tag="sxbc")
            nc.vector.tensor_tensor(sxbc, conv, xsg, Alu.mult)
            # X token-major bf16 [T, 256] (transpose the two x tiles)
            X_tm = sb(pact, [T, XC], wdt, tag="X_tm")
            for i in range(2):
                pt = ppt.tile([T, 128], f32, tag="t256", name="t256")
                nc.tensor.transpose(pt, sxbc[:, i, :], idn)
                nc.vector.tensor_copy(out=X_tm[:, i * 128:(i + 1) * 128],
                                      in_=pt)
            # GT[s,t] = B^T C (shared across heads)
            ps_gt = ppt.tile([T, T], f32, tag="t256", name="t256")
            nc.tensor.matmul(ps_gt, sxbc[:, 2, :], sxbc[:, 3, :],
                             start=True, stop=True)
            GTs = sb(psm, [T, T], f32, tag="GT")
            nc.vector.tensor_copy(out=GTs, in_=ps_gt)
            # dt path
            dte = sb(psm, [T, HC], f32, tag="dte")
            nc.scalar.activation(dte, dtt, Act.Exp)
            dt_tp = sb(psm, [T, HC], f32, tag="dt_tp")
            nc.scalar.activation(dt_tp, dte, Act.Ln, bias=1.0)
            logdA = sb(psm, [T, HC], f32, tag="logdA")
            nc.vector.tensor_tensor(logdA, dt_tp, cl["abc"], Alu.mult)
            ps_n = ppt.tile([T, HC], f32, tag="t256", name="t256")
            nc.tensor.matmul(ps_n, negut, logdA, start=True, stop=True)
            n_tp = sb(psm, [T, HC], f32, tag="n_tp")         # -cumsum(logdA)
            nc.vector.tensor_copy(out=n_tp, in_=ps_n)
            ps_nf = ppt.tile([HC, T], f32, tag="t256", name="t256")
            nc.tensor.transpose(ps_nf, n_tp, idn[0:T, 0:T])
            n_fm = sb(psm, [HC, T], wdt, tag="n_fm")
            nc.vector.tensor_copy(out=n_fm, in_=ps_nf)
            # batched SSD: Dm[s, h*T+t] = n[s,h] - n[t,h] - maskpos[s,t]
            n_blkneg = sb(psm, [HC, HC * T], wdt, tag="nblk")
            n_rep = bass.AP(tensor=n_fm.tensor, offset=n_fm.offset,
                            ap=[list(n_fm.ap[0]), [0, HC], [1, T]])
            nc.vector.tensor_tensor(n_blkneg, n_rep, blkneg, Alu.mult)
            ps_dm = ppt.tile([T, HC * T], f32, tag="t256", name="t256")
            nc.tensor.matmul(ps_dm, n_fm, blkpos, start=True, stop=False)
            nc.tensor.matmul(ps_dm, ones4, n_blkneg, start=False, stop=False)
            nc.tensor.matmul(ps_dm, negmaskT, blk32, start=False, stop=True)
            E = sb(psm, [T, HC * T], f32, tag="E")
            nc.scalar.activation(E, ps_dm, Act.Exp)
            gt_rep = bass.AP(tensor=GTs.tensor, offset=GTs.offset,
                             ap=[list(GTs.ap[0]), [0, HC], [1, T]])
            dt_b = bass.AP(tensor=dt_tp.tensor, offset=dt_tp.offset,
                           ap=[list(dt_tp.ap[0]), [1, HC], [0, T]])
            GTD = sb(psm, [T, HC * T], f32, tag="GTD")
            nc.vector.tensor_tensor(GTD, gt_rep, dt_b, Alu.mult)
            M2a = sb(psm, [T, HC * T], f32, tag="M2a")
            nc.vector.tensor_tensor(M2a, E, GTD, Alu.mult)
            M2 = sb(psm, [T, HC * T], wdt, tag="M2")
            nc.vector.tensor_tensor(M2, M2a, cl["ddiag"], Alu.add)
            # Y feature-major: heads 0,1 -> ps_y0 [128,T], heads 2,3 -> ps_y1
            ps_y0 = pps.tile([128, T], f32, tag="t128", name="t128")
            ps_y1 = pps.tile([128, T], f32, tag="t128", name="t128")
            for h in range(HC):
                dst = ps_y0 if h < 2 else ps_y1
                hh = h % 2
                nc.tensor.matmul(dst[64 * hh:64 * (hh + 1), :],
                                 X_tm[0:T, 64 * h:64 * h + 64],
                                 M2[:, T * h:T * h + T],
                                 start=True, stop=True)
            # gate (feature-major, no transposes) + sumsq + Wout partial
            ygb = sb(pact, [128, 2, T], wdt, tag="ygb")
            nc.vector.tensor_tensor(ygb[:, 0, :], ps_y0, szf[:, 0:T], Alu.mult)
            nc.vector.tensor_tensor(ygb[:, 1, :], ps_y1, szf[:, T:2 * T],
                                    Alu.mult)
            sq = sb(pact, [128, 2 * T], f32, tag="sq")
            ygb_flat = bass.AP(tensor=ygb.tensor, offset=ygb.offset,
                               ap=[list(ygb.ap[0]), [1, 2 * T]])
            nc.scalar.activation(sq, ygb_flat, Act.Square)
            ps_ss = ppt.tile([T, 1], f32, tag="t256", name="t256")
            nc.tensor.matmul(ps_ss, sq[:, 0:T], onescol, start=True, stop=False)
            nc.tensor.matmul(ps_ss, sq[:, T:2 * T], onescol,
                             start=False, stop=True)
            comb = sb(pact, [128, 257], wdt, tag="comb")
            nc.vector.memset(comb[0:128, 256:257], 0.0)
            nc.vector.tensor_copy(out=comb[0:T, 256:257], in_=ps_ss)
            ps_p = pyacc.tile([128, 8, T], f32, tag="yacc", name="yacc")
            arin = dram.tile([128, 257], wdt, tag="arin", name="arin")
            for mt in range(8):
                for ki in range(2):
                    nc.tensor.matmul(ps_p[:, mt, :], woutsb[:, ki, mt, :],
                                     ygb[:, ki, :],
                                     start=(ki == 0), stop=(ki == 1))
                if mt == 3:
                    # first half streams to DRAM while the rest computes
                    nc.vector.tensor_copy(out=comb[:, 0:128],
                                          in_=ps_p[:, 0:4, :])
                    nc.sync.dma_start(out=arin[:][:, 0:128],
                                      in_=comb[:, 0:128])
            nc.vector.tensor_copy(out=comb[:, 128:256], in_=ps_p[:, 4:8, :])
            # fused AllReduce (bf16): [128, 256] partial + [32] sumsq
            i_arin = nc.sync.dma_start(out=arin[:][:, 128:257],
                                       in_=comb[:, 128:257])
            arout = dram.tile([128, 257], wdt, tag="arout", name="arout")
            _collective("AllReduce", Alu.add, [arin[:].opt()], [arout[:].opt()])
            keep_warm(36 if l < NL - 1 else 40, i_arin, f"AR{l}")
            comb2 = sb(pact, [128, 257], wdt, tag="comb2")
            aro = arout[:]
            nc.sync.dma_start(out=comb2[:, 256:257], in_=aro[:, 256:257])
            nc.sync.dma_start(out=comb2[:, 0:256], in_=aro[:, 0:256])

        # final norm scale r3 for stage C input
        s_col = sb(pact, [T, 1], f32, tag="s_col")
        nc.scalar.activation(s_col, comb2[0:T, 256:257], Act.Ln,
                             bias=eps_t[0:T], scale=1.0 / DI)
        r_col = sb(pact, [T, 1], f32, tag="r_col")
        nc.scalar.activation(r_col, s_col, Act.Exp, scale=-0.5)
        rt_ps = ppt.tile([1, T], f32, tag="t256", name="t256")
        nc.tensor.transpose(rt_ps, r_col, idn[0:T, 0:T])
        r_row = sb(pact, [1, T], f32, tag="r_row")
        nc.vector.tensor_copy(out=r_row, in_=rt_ps)
        ps_r = pps.tile([128, T], f32, tag="t128", name="t128")
        nc.tensor.matmul(ps_r, ones1, r_row, start=True, stop=True)
        r_bc = sb(pact, [128, T], f32, tag="r_bc")
        nc.vector.tensor_copy(out=r_bc, in_=ps_r)

        # ---- stage C: g = relu(w3 @ flat + b3) -> AllGather
        # uT8[p, half, j] = u chunk (half*128 + j), fp8 pairs (j, j+128)
        uT8 = sb(pact, [128, 2, 128], dt8, tag="uT8")
        uT8_v = bass.AP(tensor=uT8.tensor, offset=uT8.offset,
                        ap=[list(uT8.ap[0]), [128, 2], [8, 16], [1, 8]])
        c2_perm = bass.AP(tensor=comb2.tensor, offset=comb2.offset,
                          ap=[list(comb2.ap[0]), [16, 2], [1, 16], [32, 8]])
        r_perm = bass.AP(tensor=r_bc.tensor, offset=r_bc.offset,
                         ap=[list(r_bc.ap[0]), [16, 2], [1, 16], [0, 8]])
        nc.vector.tensor_tensor(uT8_v, c2_perm, r_perm, Alu.mult)
        ps_g = pmv.tile([1, 512], f32, tag="mv", name="mv")
        for d in range(8):
            w3sb = sb(pw3, [128, 16, 2, 128], dt8, tag="w3")
            i_d = nc.scalar.dma_start(out=w3sb, in_=w3t_d[d])
            add_dep_helper(i_d.ins, i_gate_a.ins, reason="prefetch w3 early")
            for jj in range(16):
                j = d * 16 + jj
                nc.tensor.matmul(ps_g[0:1, 0:128], uT8[:, :, j:j + 1],
                                 w3sb[:, jj],
                                 start=(j == 0), stop=(j == 127), perf_mode=DR)
        g_tmp = sb(pact1, [1, 128], f32, tag="g_tmp")
        nc.vector.tensor_tensor(g_tmp, ps_g[0:1, 0:128], b3row, Alu.add)
        g_act = sb(pact, [1, 128], wdt, tag="g_act")
        nc.scalar.activation(g_act, g_tmp, Act.Relu)
        gin = dram.tile([1, 128], wdt, tag="gin", name="gin")
        i_gin = nc.sync.dma_start(out=gin[:], in_=g_act, single_packet=True)
        gout = dram.tile([NCORES, 128], wdt, tag="gout", name="gout")
        _collective("AllGather", Alu.bypass, [gin[:].opt()], [gout[:].opt()])
        keep_warm(30, i_gin, "AG2")

        # ---- stage D: out = sigmoid(w4 @ g + b4)
        g_all = sb(pact, [128, NCORES], wdt, tag="g_all")
        nc.sync.dma_start(out=g_all, in_=gout[:].rearrange("j p -> p j"))
        g8c = sb(pact, [128, 2, 16], dt8, tag="g8c")
        g_all_pair = bass.AP(tensor=g_all.tensor, offset=g_all.offset,
                             ap=[list(g_all.ap[0]), [1, 2], [2, 4]])
        nc.vector.tensor_copy(out=g8c[:, :, 0:4], in_=g_all_pair)
        o_tmp = sb(pact1, [1, 2048], wdt, tag="o_tmp")
        for nt in range(4):
            w4sb = sb(pw4, [128, 4, 2, 512], dt8, tag="w4")
            i_d = nc.scalar.dma_start(out=w4sb, in_=w4t_d[nt])
            add_dep_helper(i_d.ins, i_gate_a.ins, reason="prefetch w4 early")
            ps = pmv.tile([1, 512], f32, tag="mv", name="mv")
            for j in range(4):
                nc.tensor.matmul(ps, g8c[:, :, j:j + 1], w4sb[:, j],
                                 start=(j == 0), stop=(j == 3), perf_mode=DR)
            nc.vector.tensor_tensor(o_tmp[0:1, nt * 512:(nt + 1) * 512], ps,
                                    b4row[0:1, nt * 512:(nt + 1) * 512], Alu.add)
        # repack [1,2048] -> [128,16] so the sigmoid ACT chain uses all lanes
        px = sb(pact, [128, 16], wdt, tag="px")
        o_sc = bass.AP(tensor=o_tmp.tensor, offset=o_tmp.offset,
                       ap=[list(o_tmp.ap[0]), [16, 128], [1, 16]])
        nc.sync.dma_start(out=px, in_=o_sc)
        oeg = sb(pact, [128, 16], f32, tag="oeg")
        nc.scalar.activation(oeg, px, Act.Exp, scale=-1.0)
        osp = sb(pact, [128, 16], f32, tag="osp")
        nc.scalar.activation(osp, oeg, Act.Ln, bias=1.0)
        out_sb = sb(pact1, [128, 16], f32, tag="out_sb")
        nc.scalar.activation(out_sb, osp, Act.Exp, scale=-1.0)
        o_dv = bass.AP(tensor=out_d.tensor, offset=out_d.offset,
                       ap=[list(out_d.ap[0]), [16, 128], [1, 16]])
        nc.sync.dma_start(out=o_dv, in_=out_sb)

    nc.compile()
    return nc


_CACHE = {}


def _get_program():
    if "nc" not in _CACHE:
        _CACHE["nc"] = _build_program(BF16)
    return _CACHE["nc"]


def kernel(**inputs):
    from concourse.bass_utils import run_bass_kernel_spmd
    nc = _get_program()
    in_maps = [_prep_core(inputs, c, BF16) for c in range(NCORES)]
    res = run_bass_kernel_spmd(nc, in_maps, core_ids=list(range(NCORES)))
    out = np.concatenate([res.results[c]["out"].ravel()
                          for c in range(NCORES)])
    return out.reshape(8, 32, 64).astype(np.float32)


if __name__ == "__main__":
    d = np.load("/tmp/inp.npz")
    inp = {k: d[k] for k in d.files}
    got = kernel(**inp)
    want = np.load("/tmp/want64.npy")
    err = np.abs(got - want) / (np.abs(want) + 1e-6)
    print(f"maxrel {err.max():.3e} mean {err.mean():.3e}")

